# revision 23
# baseline (speedup 1.0000x reference)
"""Trainium2 Bass kernel for nn_AutomatonPELayer (n=512, k=16, d=512).

Math: the reference solves B x = tile(p) with B = I - kron(shift, T),
which is block upper-bidiagonal => stacked[i] = s_{n-1-i} where
s_m = sum_{j<=m} T^j p.  In homogeneous coordinates s-hat_m = [s_m; 1],
the prefix satisfies s-hat_{w+m} = M_w s-hat_m with
M_w = [[T^w, s_{w-1}], [0, 1]], and M_a M_b = M_{a+b}.  So a log-depth
doubling scan on the 17x17 M (tracking both M and Q = M^T, since the PE
computes lhsT.T @ rhs) builds S64 = [s-hat_0 .. s-hat_63] in 6 rounds.
Core with jump q then applies M_{64q} = M_256^bb * M_{64 ba} (q = ba+4bb)
as two data-selected matmuls: the selector matrices are 0/1 masks sent
from the host (layout-only), applied with copy_predicated onto
identity-prefilled tiles, so all 8 cores run one instruction stream.
The projection pe-block = Cb^T Wb is two float32r matmuls with 256-wide
moving dim (1 cycle/row vs fp32's 4); the homogeneous ones-row provides
the bias for free.  Host work is layout-only: M1/Q1 assembly, identity /
0-1 mask tiles, W^T|b concat, row-reversal on output assembly.

Hardware notes (from trace analysis of earlier revisions):
  - DMA rows must be <= 2048B or the transfer serializes on one queue
    (~100ns/descriptor); inputs are split into two DMAs of 548B/2048B
    rows.  Each dma_start also costs ~700ns of Sync-engine descriptor
    generation, so there are exactly two input DMAs and one output DMA.
  - fp32 matmuls run LOW+HIGH double passes (4 cyc/row); float32r with
    moving dim >= 256 runs single pass.  The scan and applies stay fp32
    for precision; the projection is float32r.  fp32r operands must come
    from fp32r-typed producers (BIR verifier), so Wb lands in an f32r
    tile via its own DMA and Cb is rounded into an f32r tile by the DVE
    copy; its 128-column zero padding is an fp32 memset + rounding copy.
  - The per-round pair copy (DVE, [Q|M] adjacent in PSUM) gates the next
    round; the S-extension copy (ACT) is off the critical path with its
    own semaphore.  Two engines never read the same PSUM tensor
    concurrently (observed hardware failure).
  - PSUM columns are never recycled, so no WAR waits.
"""

import numpy as np

N = 512  # sentence length handled by the device kernel
K = 16  # num states
H = K + 1  # homogeneous dim
D = 512  # embed dim
NCORES = 8
PPOS = N // NCORES  # positions per core (64)

# tAll column map
C_PAIR0 = 0  # [Q1 | M1]
C_S = 34  # s-hat_0 at col 34, S grows to col 98
C_JAT = 98
C_JBT = 115
C_BA1 = 132
C_BA2 = 149
C_BA3 = 166
C_BB = 183
C_PAIRS = 200  # pair_r (r=1..7) at 200+34(r-1), pair8 at 438
NCOL_IN0 = 35  # seed DMA: cols 0:35 (pair0 + s-hat_0, 140 B/row)
C_REST = 98  # second DMA: cols 98:200 (ids + masks)
NCOL_RST = 102
NCOL_ALL = 472

_NC_CACHE = {}

VARIANT = "raw"

# Set by an external harness to capture a profile; grading path leaves these.
TRACE = False
LAST_RESULT = None


def _host_fallback(p, T, W, b, n):
    # Closed-form reference for shapes the compiled kernel doesn't handle.
    p = p.reshape(-1).astype(np.float64)
    T = T.astype(np.float64)
    k = p.shape[0]
    stacked = np.zeros((n, k), dtype=np.float64)
    acc = np.zeros(k, dtype=np.float64)
    for i in range(n - 1, -1, -1):
        acc = p + (T @ acc if i < n - 1 else 0.0)
        stacked[i] = acc
    pe = stacked @ W.astype(np.float64).T + b.astype(np.float64)
    return pe.astype(np.float32)


def _build_nc_raw():
    """Hand-scheduled Bacc build: no TileContext, explicit semaphores.

    Engine streams:
      SP  : dma seed | dma masks | dma Wb | dma out lo | wait out
      PE  : 2 warmup MMs | 6 rounds of (mmQ, mmM, mmS) | mmQ7, mmM7 |
            mm8a, mm8b | mmA | mmB | mmP0, mmP1
      DVE : memset junk/pad | cpQM 1..7 | bA1 bA2 | bA3 bB (PSUM reads) |
            cpCa | cpCb (fp32r cast) | cast out hi
      ACT : cpS 1..6 | cast out lo | dma out hi
    """
    from contextlib import ExitStack

    import concourse.mybir as mybir
    from concourse import bacc

    f32 = mybir.dt.float32
    f32r = mybir.dt.float32r
    nc = bacc.Bacc("TRN2", target_bir_lowering=False)

    dIn0 = nc.dram_tensor("inp0", [H, NCOL_IN0], f32, kind="ExternalInput")
    dIn1 = nc.dram_tensor("inp1", [H, NCOL_RST], f32, kind="ExternalInput")
    dIn2 = nc.dram_tensor("inp2", [H, D], f32r, kind="ExternalInput")
    dOut = nc.dram_tensor("out", [PPOS, D], mybir.dt.bfloat16, kind="ExternalOutput")

    with ExitStack() as ctx:
        tAll = ctx.enter_context(nc.sbuf_tensor("tAll", [H, NCOL_ALL], f32))
        tWb = ctx.enter_context(nc.sbuf_tensor("tWb", [H, D], f32r))
        tCa = ctx.enter_context(nc.sbuf_tensor("tCa", [H, PPOS], f32))
        tZ = ctx.enter_context(nc.sbuf_tensor("tZ", [H, PPOS], f32))
        tJk = ctx.enter_context(nc.sbuf_tensor("tJk", [128, 640], mybir.dt.bfloat16))
        tCbP = ctx.enter_context(nc.sbuf_tensor("tCbP", [H, 128], f32r))
        tOut = ctx.enter_context(
            nc.sbuf_tensor("tOut", [PPOS, D], mybir.dt.bfloat16)
        )

        def psb(name, shape):
            return ctx.enter_context(nc.psum_tensor(name, shape, f32))

        psQMall = psb("psQM", [H, 340])
        psQM = psQMall[:, 0 : 34 * 7]
        psQ256 = psQMall[:, 238:255]
        psC3 = psQMall[:, 272:336]
        psSall = psb("psS", [H, 127])
        psS = psSall[:, 0:63]
        psC2 = psSall[:, 63:127]
        psC1 = psb("psC", [H, PPOS])
        psOa = psb("psOa", [128, 256])
        psOb = psb("psOb", [128, 256])
        psJa = psb("psJa", [128, 512])
        psJb = psb("psJb", [128, 512])

        dmaIn = nc.alloc_semaphore("dmaIn")
        dmaInR = nc.alloc_semaphore("dmaInR")
        dmaIn2 = nc.alloc_semaphore("dmaIn2")
        dmaO = nc.alloc_semaphore("dmaO")
        pe = nc.alloc_semaphore("peS")
        qmP = nc.alloc_semaphore("qmP")  # DVE stream
        qmS = nc.alloc_semaphore("qmS")  # ACT scan copies
        outS = nc.alloc_semaphore("outS")

        npe = [0]  # pe count after each PE instruction
        nqp = [0]  # qmP (DVE) count
        nqs = [0]  # qmS (ACT) count

        def pe_inc(instr):
            npe[0] += 1
            return instr.then_inc(pe, 1)

        def qp_inc(instr):
            nqp[0] += 1
            return instr.then_inc(qmP, 1)

        def qs_inc(instr):
            nqs[0] += 1
            return instr.then_inc(qmS, 1)

        def pair(r):
            # [Q_{2^r} | M_{2^r}] columns in tAll
            if r == 0:
                return tAll[:, C_PAIR0 : C_PAIR0 + 34]
            return tAll[:, C_PAIRS + 34 * (r - 1) : C_PAIRS + 34 * r]

        # --- input DMAs (tiny seed first so the scan starts earliest;
        # masks/ids next; Wb last, only needed at the projection) ---
        nc.sync.dma_start(out=tAll[:, 0:NCOL_IN0], in_=dIn0[:]).then_inc(dmaIn, 16)
        nc.sync.dma_start(
            out=tAll[:, C_REST : C_REST + NCOL_RST], in_=dIn1[:]
        ).then_inc(dmaInR, 16)
        nc.sync.dma_start(out=tWb[:], in_=dIn2[:]).then_inc(dmaIn2, 16)

        # --- PE warmup: bf16 junk matmuls during the input-DMA flight keep
        # the HAM activity monitor busy so later matmuls run at 2.4 GHz.
        # Separate PSUM tensors -> no WAW -> no inter-MM semaphores. ---
        qp_inc(nc.vector.memset(tJk[:], 0.0))
        mjk = nqp[0]
        nc.tensor.matmul(
            psJa[:], lhsT=tJk[:, 0:128], rhs=tJk[:, 128:640],
            start=True, stop=True,
        )._wait_ge(qmP, mjk)
        nc.tensor.matmul(
            psJb[:], lhsT=tJk[:, 0:128], rhs=tJk[:, 128:640],
            start=True, stop=True,
        )._wait_ge(qmP, mjk)

        # --- DVE: zero-pad the fp32r Cb columns 64:128 once, up front
        # (fp32 memset into scratch, then a rounding copy into the f32r
        # tile; a direct f32r memset fails the ISA check) ---
        qp_inc(nc.vector.memset(tZ[:], 0.0))
        mz = nqp[0]
        qp_inc(nc.vector.tensor_copy(out=tCbP[:, PPOS:128], in_=tZ[:])._wait_ge(qmP, mz))

        # --- scan rounds r=1..6 (w = 2^(r-1)) ---
        cpq_at = {}  # round -> qmP count of its pair copy
        cps_at = {}  # round -> qmS count of its S copy
        mm_at = {}  # tag -> pe count
        for r in range(1, 7):
            w = 1 << (r - 1)
            prev = pair(r - 1)
            tQ = prev[:, 0:17]
            tM = prev[:, 17:34]
            po = 34 * (r - 1)
            soff = w - 1
            mq = pe_inc(
                nc.tensor.matmul(
                    psQM[:, po : po + 17], lhsT=tM, rhs=tQ, start=True, stop=True
                )
            )
            mm = pe_inc(
                nc.tensor.matmul(
                    psQM[:, po + 17 : po + 34], lhsT=tQ, rhs=tM, start=True, stop=True
                )
            )
            mm_at[("m", r)] = npe[0]
            ms = pe_inc(
                nc.tensor.matmul(
                    psS[:, soff : soff + w],
                    lhsT=tQ,
                    rhs=tAll[:, C_S : C_S + w],
                    start=True,
                    stop=True,
                )
            )
            mm_at[("s", r)] = npe[0]
            if r == 1:
                mq._wait_ge(dmaIn, 16)
                mm._wait_ge(dmaIn, 16)
                ms._wait_ge(dmaIn, 16)
            else:
                # mmM/mmS inherit the pair dependency through PE program
                # order after mmQ's wait; mmS only needs the S-copy edge
                mq._wait_ge(qmP, cpq_at[r - 1])
                ms._wait_ge(qmS, cps_at[r - 1])
            qp_inc(
                nc.vector.tensor_copy(
                    out=pair(r)[:], in_=psQM[:, po : po + 34]
                )._wait_ge(pe, mm_at[("m", r)])
            )
            cpq_at[r] = nqp[0]
            qs_inc(
                nc.scalar.copy(
                    out=tAll[:, C_S + w : C_S + 2 * w], in_=psS[:, soff : soff + w]
                )._wait_ge(pe, mm_at[("s", r)])
            )
            cps_at[r] = nqs[0]

        # --- r7: [Q128 | M128] (no S extension) ---
        p6 = pair(6)
        pe_inc(
            nc.tensor.matmul(
                psQM[:, 204:221], lhsT=p6[:, 17:34], rhs=p6[:, 0:17],
                start=True, stop=True,
            )._wait_ge(qmP, cpq_at[6])
        )
        pe_inc(
            nc.tensor.matmul(
                psQM[:, 221:238], lhsT=p6[:, 0:17], rhs=p6[:, 17:34],
                start=True, stop=True,
            )
        )
        mm_at["r7"] = npe[0]

        # --- binary-decomposed jump: M_{64q} = M256^b2 M128^b1 M64^b0.
        # apply-k's matmul hides in the PE idle window of the next power
        # round, so only apply-3 + projection trail the power chain. ---
        tJ1 = tAll[:, C_JAT : C_JAT + 17]
        tJ2 = tAll[:, C_JBT : C_JBT + 17]
        tJ3 = tAll[:, C_BA3 : C_BA3 + 17]

        # blend1 (J1 in {I, M64}; data = Q64 from pair6)
        b1 = nc.vector.copy_predicated(
            out=tJ1,
            mask=tAll[:, C_BA1 : C_BA1 + 17].bitcast(mybir.dt.uint32),
            data=p6[:, 0:17],
        )._wait_ge(qmP, cpq_at[6])
        b1.wait_op(dmaInR, 16, "sem-ge", check=False)
        qp_inc(b1)
        bl1 = nqp[0]

        # apply-1: C1 = J1 S64 (runs in r7's shadow)
        a1 = pe_inc(
            nc.tensor.matmul(
                psC1[:], lhsT=tJ1, rhs=tAll[:, C_S : C_S + PPOS],
                start=True, stop=True,
            )
        )
        a1._wait_ge(qmP, bl1)
        a1.wait_op(qmS, cps_at[6], "sem-ge", check=False)  # S64 complete
        mm_at["a1"] = npe[0]

        qp_inc(
            nc.vector.tensor_copy(out=pair(7)[:], in_=psQM[:, 204:238])._wait_ge(
                pe, mm_at["r7"]
            )
        )
        cpq7 = nqp[0]

        # blend2 (J2 in {I, M128}; data = Q128 from pair7)
        qp_inc(
            nc.vector.copy_predicated(
                out=tJ2,
                mask=tAll[:, C_BA2 : C_BA2 + 17].bitcast(mybir.dt.uint32),
                data=pair(7)[:, 0:17],
            )._wait_ge(qmP, cpq7)
        )
        bl2 = nqp[0]
        # cpC1 (DVE): C1 -> SBUF; ordered after blend2 so apply-2's single
        # qmP wait covers both
        qp_inc(nc.vector.tensor_copy(out=tCa[:], in_=psC1[:])._wait_ge(pe, mm_at["a1"]))
        cpc1 = nqp[0]

        # --- r8: Q256 = Q128 Q128 ---
        mq256 = pe_inc(
            nc.tensor.matmul(
                psQ256[:], lhsT=pair(7)[:, 17:34], rhs=pair(7)[:, 0:17],
                start=True, stop=True,
            )._wait_ge(qmP, cpq7)
        )
        mm_at["r8"] = npe[0]

        # apply-2: C2 = J2 C1 (runs in r8's shadow)
        pe_inc(
            nc.tensor.matmul(
                psC2[:], lhsT=tJ2, rhs=tCa[:], start=True, stop=True
            )._wait_ge(qmP, cpc1)
        )
        mm_at["a2"] = npe[0]

        # blend3 (J3 in {I, M256}; data = Q256 read straight from PSUM)
        qp_inc(
            nc.vector.copy_predicated(
                out=tJ3,
                mask=tAll[:, C_BB : C_BB + 17].bitcast(mybir.dt.uint32),
                data=psQ256[:],
            )._wait_ge(pe, mm_at["r8"])
        )
        bl3 = nqp[0]
        # cpC2 (ACT): C2 -> SBUF
        qs_inc(nc.scalar.copy(out=tZ[:], in_=psC2[:])._wait_ge(pe, mm_at["a2"]))
        cpc2 = nqs[0]

        # apply-3: C3 = J3 C2
        a3 = pe_inc(
            nc.tensor.matmul(psC3[:], lhsT=tJ3, rhs=tZ[:], start=True, stop=True)
        )
        a3._wait_ge(qmP, bl3)
        a3.wait_op(qmS, cpc2, "sem-ge", check=False)
        mm_at["a3"] = npe[0]
        # DVE cast rounds C3 into the fp32r projection operand
        qp_inc(
            nc.vector.tensor_copy(out=tCbP[:, 0:PPOS], in_=psC3[:])._wait_ge(
                pe, mm_at["a3"]
            )
        )
        cpCb = nqp[0]

        # --- projection: psO = CbPad^T Wb (float32r, 256-wide moving) ---
        mp0 = pe_inc(
            nc.tensor.matmul(
                psOa[:], lhsT=tCbP[:], rhs=tWb[:, 0:256], start=True, stop=True
            )
        )
        mp0._wait_ge(qmP, cpCb)
        mp0.wait_op(dmaIn2, 16, "sem-ge", check=False)  # Wb present
        mm_at["p0"] = npe[0]
        pe_inc(
            nc.tensor.matmul(
                psOb[:], lhsT=tCbP[:], rhs=tWb[:, 256:512], start=True, stop=True
            )
        )
        mm_at["p1"] = npe[0]

        # --- output copies (separate PSUM tensors per engine) + DMA ---
        nc.scalar.copy(out=tOut[:, 0:256], in_=psOa[0:PPOS, :])._wait_ge(
            pe, mm_at["p0"]
        ).then_inc(outS, 1)
        nc.vector.tensor_copy(out=tOut[:, 256:512], in_=psOb[0:PPOS, :])._wait_ge(
            pe, mm_at["p1"]
        ).then_inc(outS, 1)
        HP = PPOS // 2
        nc.sync.dma_start(out=dOut[0:HP, :], in_=tOut[0:HP, :])._wait_ge(
            outS, 2
        ).then_inc(dmaO, 16)
        nc.scalar.dma_start(out=dOut[HP:PPOS, :], in_=tOut[HP:PPOS, :])._wait_ge(
            outS, 2
        ).then_inc(dmaO, 16)
        nc.sync.wait_ge(dmaO, 32)

    nc.compile()
    return nc


def get_nc():
    key = VARIANT
    if key not in _NC_CACHE:
        _NC_CACHE[key] = _build_nc_raw()
    return _NC_CACHE[key]


def make_in_maps(pos_initial, pos_transition, W, b):
    T = np.ascontiguousarray(pos_transition, dtype=np.float32)
    p = np.asarray(pos_initial, dtype=np.float32).reshape(K)

    M1 = np.zeros((H, H), dtype=np.float32)
    M1[0:K, 0:K] = T
    M1[0:K, K] = p
    M1[K, K] = 1.0
    I17 = np.eye(H, dtype=np.float32)
    ones = np.ones((H, H), dtype=np.float32)
    zeros = np.zeros((H, H), dtype=np.float32)
    wb = np.concatenate(
        [W.T.astype(np.float32), b.reshape(1, -1).astype(np.float32)], axis=0
    )
    s0 = np.concatenate([p, [1.0]]).astype(np.float32)

    in_maps = []
    for c in range(NCORES):
        q = (NCORES - 1) - c
        inp0 = np.zeros((H, NCOL_IN0), dtype=np.float32)
        inp0[:, 0:17] = M1.T
        inp0[:, 17:34] = M1
        inp0[:, 34] = s0
        inp1 = np.zeros((H, NCOL_RST), dtype=np.float32)
        inp1[:, C_JAT - C_REST : C_JAT - C_REST + 17] = I17
        inp1[:, C_JBT - C_REST : C_JBT - C_REST + 17] = I17
        inp1[:, C_BA3 - C_REST : C_BA3 - C_REST + 17] = I17  # J3 identity
        inp1[:, C_BA1 - C_REST : C_BA1 - C_REST + 17] = ones if q & 1 else zeros
        inp1[:, C_BA2 - C_REST : C_BA2 - C_REST + 17] = ones if q & 2 else zeros
        inp1[:, C_BB - C_REST : C_BB - C_REST + 17] = ones if q & 4 else zeros
        in_maps.append(
            {
                "inp0": np.ascontiguousarray(inp0),
                "inp1": np.ascontiguousarray(inp1),
                "inp2": np.ascontiguousarray(wb),
            }
        )
    return in_maps


def assemble_output(per_core_results):
    out = np.empty((N, D), dtype=np.float32)
    for c in range(NCORES):
        arr = np.asarray(per_core_results[c]["out"])  # [64, 512]
        out[PPOS * c : PPOS * (c + 1), :] = arr[::-1, :]
    return out


def kernel(**inputs):
    pos_initial = np.asarray(inputs["pos_initial"], dtype=np.float32)
    pos_transition = np.asarray(inputs["pos_transition"], dtype=np.float32)
    W = np.asarray(inputs["W"], dtype=np.float32)
    b = np.asarray(inputs["b"], dtype=np.float32)
    n = int(inputs["sentence_len"])

    if n != N or pos_initial.shape[0] != K or W.shape != (D, K):
        return _host_fallback(pos_initial, pos_transition, W, b, n)

    from concourse.bass_utils import run_bass_kernel_spmd

    nc = get_nc()
    in_maps = make_in_maps(pos_initial, pos_transition, W, b)
    kwargs = {"trace": True} if TRACE else {}
    res = run_bass_kernel_spmd(nc, in_maps, core_ids=list(range(NCORES)), **kwargs)
    global LAST_RESULT
    LAST_RESULT = res
    return assemble_output(res.results)


if __name__ == "__main__":
    rng = np.random.default_rng(0)
    p = rng.normal(size=(K, 1)).astype(np.float32)
    A = rng.normal(size=(K, K)).astype(np.float32)
    q, r = np.linalg.qr(A)
    T = (q * np.sign(np.diag(r))[None, :]).astype(np.float32)
    W = rng.uniform(-0.25, 0.25, size=(D, K)).astype(np.float32)
    b = rng.uniform(-0.25, 0.25, size=(D,)).astype(np.float32)
    ref = _host_fallback(p, T, W, b, N)
    act = kernel(pos_initial=p, pos_transition=T, W=W, b=b, sentence_len=N)
    err = np.abs(act - ref).max() / np.abs(ref).max()
    print("max rel err vs host closed form:", err)


# revision 24
# speedup vs baseline: 1.0080x; 1.0080x over previous
"""Trainium2 Bass kernel for nn_AutomatonPELayer (n=512, k=16, d=512).

Math: the reference solves B x = tile(p) with B = I - kron(shift, T),
which is block upper-bidiagonal => stacked[i] = s_{n-1-i} where
s_m = sum_{j<=m} T^j p.  In homogeneous coordinates s-hat_m = [s_m; 1],
the prefix satisfies s-hat_{w+m} = M_w s-hat_m with
M_w = [[T^w, s_{w-1}], [0, 1]], and M_a M_b = M_{a+b}.  So a log-depth
doubling scan on the 17x17 M (tracking both M and Q = M^T, since the PE
computes lhsT.T @ rhs) builds S64 = [s-hat_0 .. s-hat_63] in 6 rounds.
Core with jump q then applies M_{64q} = M_256^bb * M_{64 ba} (q = ba+4bb)
as two data-selected matmuls: the selector matrices are 0/1 masks sent
from the host (layout-only), applied with copy_predicated onto
identity-prefilled tiles, so all 8 cores run one instruction stream.
The projection pe-block = Cb^T Wb is two float32r matmuls with 256-wide
moving dim (1 cycle/row vs fp32's 4); the homogeneous ones-row provides
the bias for free.  Host work is layout-only: M1/Q1 assembly, identity /
0-1 mask tiles, W^T|b concat, row-reversal on output assembly.

Hardware notes (from trace analysis of earlier revisions):
  - DMA rows must be <= 2048B or the transfer serializes on one queue
    (~100ns/descriptor); inputs are split into two DMAs of 548B/2048B
    rows.  Each dma_start also costs ~700ns of Sync-engine descriptor
    generation, so there are exactly two input DMAs and one output DMA.
  - fp32 matmuls run LOW+HIGH double passes (4 cyc/row); float32r with
    moving dim >= 256 runs single pass.  The scan and applies stay fp32
    for precision; the projection is float32r.  fp32r operands must come
    from fp32r-typed producers (BIR verifier), so Wb lands in an f32r
    tile via its own DMA and Cb is rounded into an f32r tile by the DVE
    copy; its 128-column zero padding is an fp32 memset + rounding copy.
  - The per-round pair copy (DVE, [Q|M] adjacent in PSUM) gates the next
    round; the S-extension copy (ACT) is off the critical path with its
    own semaphore.  Two engines never read the same PSUM tensor
    concurrently (observed hardware failure).
  - PSUM columns are never recycled, so no WAR waits.
"""

import numpy as np

N = 512  # sentence length handled by the device kernel
K = 16  # num states
H = K + 1  # homogeneous dim
D = 512  # embed dim
NCORES = 8
PPOS = N // NCORES  # positions per core (64)

# tAll column map
C_PAIR0 = 0  # [Q1 | M1]
C_S = 34  # s-hat_0 at col 34, S grows to col 98
C_JAT = 98
C_JBT = 115
C_BA1 = 132
C_BA2 = 149
C_BA3 = 166
C_BB = 183
C_PAIRS = 200  # pair_r (r=1..7) at 200+34(r-1), pair8 at 438
NCOL_IN0 = 35  # seed DMA: cols 0:35 (pair0 + s-hat_0, 140 B/row)
C_REST = 98  # second DMA: cols 98:200 (ids + masks)
NCOL_RST = 102
NCOL_ALL = 472

_NC_CACHE = {}

VARIANT = "raw"

# Set by an external harness to capture a profile; grading path leaves these.
TRACE = False
LAST_RESULT = None


def _host_fallback(p, T, W, b, n):
    # Closed-form reference for shapes the compiled kernel doesn't handle.
    p = p.reshape(-1).astype(np.float64)
    T = T.astype(np.float64)
    k = p.shape[0]
    stacked = np.zeros((n, k), dtype=np.float64)
    acc = np.zeros(k, dtype=np.float64)
    for i in range(n - 1, -1, -1):
        acc = p + (T @ acc if i < n - 1 else 0.0)
        stacked[i] = acc
    pe = stacked @ W.astype(np.float64).T + b.astype(np.float64)
    return pe.astype(np.float32)


def _build_nc_raw():
    """Hand-scheduled Bacc build: no TileContext, explicit semaphores.

    Engine streams:
      SP  : dma seed | dma masks | dma Wb | dma out lo | wait out
      PE  : 2 warmup MMs | 6 rounds of (mmQ, mmM, mmS) | mmQ7, mmM7 |
            mm8a, mm8b | mmA | mmB | mmP0, mmP1
      DVE : memset junk/pad | cpQM 1..7 | bA1 bA2 | bA3 bB (PSUM reads) |
            cpCa | cpCb (fp32r cast) | cast out hi
      ACT : cpS 1..6 | cast out lo | dma out hi
    """
    from contextlib import ExitStack

    import concourse.mybir as mybir
    from concourse import bacc

    f32 = mybir.dt.float32
    f32r = mybir.dt.float32r
    nc = bacc.Bacc("TRN2", target_bir_lowering=False)

    dIn0 = nc.dram_tensor("inp0", [H, NCOL_IN0], f32, kind="ExternalInput")
    dIn1 = nc.dram_tensor("inp1", [H, NCOL_RST], f32, kind="ExternalInput")
    dIn2 = nc.dram_tensor("inp2", [H, D], f32r, kind="ExternalInput")
    dOut = nc.dram_tensor("out", [PPOS, D], mybir.dt.bfloat16, kind="ExternalOutput")

    with ExitStack() as ctx:
        tAll = ctx.enter_context(nc.sbuf_tensor("tAll", [H, NCOL_ALL], f32))
        tWb = ctx.enter_context(nc.sbuf_tensor("tWb", [H, D], f32r))
        tCa = ctx.enter_context(nc.sbuf_tensor("tCa", [H, PPOS], f32))
        tZ = ctx.enter_context(nc.sbuf_tensor("tZ", [H, PPOS], f32))
        tJk = ctx.enter_context(nc.sbuf_tensor("tJk", [128, 640], mybir.dt.bfloat16))
        tCbP = ctx.enter_context(nc.sbuf_tensor("tCbP", [H, 128], f32r))
        tOut = ctx.enter_context(
            nc.sbuf_tensor("tOut", [PPOS, D], mybir.dt.bfloat16)
        )

        def psb(name, shape):
            return ctx.enter_context(nc.psum_tensor(name, shape, f32))

        psQMall = psb("psQM", [H, 340])
        psQM = psQMall[:, 0 : 34 * 7]
        psQ256 = psQMall[:, 238:255]
        psC3 = psQMall[:, 272:336]
        psSall = psb("psS", [H, 127])
        psS = psSall[:, 0:63]
        psC2 = psSall[:, 63:127]
        psC1 = psb("psC", [H, PPOS])
        psOa = psb("psOa", [128, 256])
        psOb = psb("psOb", [128, 256])
        psJa = psb("psJa", [128, 512])
        psJb = psb("psJb", [128, 512])

        dmaIn = nc.alloc_semaphore("dmaIn")
        dmaInR = nc.alloc_semaphore("dmaInR")
        dmaIn2 = nc.alloc_semaphore("dmaIn2")
        dmaO = nc.alloc_semaphore("dmaO")
        pe = nc.alloc_semaphore("peS")
        qmP = nc.alloc_semaphore("qmP")  # DVE stream
        qmS = nc.alloc_semaphore("qmS")  # ACT scan copies
        outS = nc.alloc_semaphore("outS")

        npe = [0]  # pe count after each PE instruction
        nqp = [0]  # qmP (DVE) count
        nqs = [0]  # qmS (ACT) count

        def pe_inc(instr):
            npe[0] += 1
            return instr.then_inc(pe, 1)

        def qp_inc(instr):
            nqp[0] += 1
            return instr.then_inc(qmP, 1)

        def qs_inc(instr):
            nqs[0] += 1
            return instr.then_inc(qmS, 1)

        def pair(r):
            # [Q_{2^r} | M_{2^r}] columns in tAll
            if r == 0:
                return tAll[:, C_PAIR0 : C_PAIR0 + 34]
            return tAll[:, C_PAIRS + 34 * (r - 1) : C_PAIRS + 34 * r]

        # --- input DMAs (tiny seed first so the scan starts earliest;
        # masks/ids next; Wb last, only needed at the projection) ---
        nc.sync.dma_start(out=tAll[:, 0:NCOL_IN0], in_=dIn0[:]).then_inc(dmaIn, 16)
        nc.sync.dma_start(
            out=tAll[:, C_REST : C_REST + NCOL_RST], in_=dIn1[:]
        ).then_inc(dmaInR, 16)
        nc.sync.dma_start(out=tWb[:], in_=dIn2[:]).then_inc(dmaIn2, 16)

        # --- PE warmup: bf16 junk matmuls during the input-DMA flight keep
        # the HAM activity monitor busy so later matmuls run at 2.4 GHz.
        # Separate PSUM tensors -> no WAW -> no inter-MM semaphores. ---
        qp_inc(nc.vector.memset(tJk[:], 0.0))
        mjk = nqp[0]
        nc.tensor.matmul(
            psJa[:], lhsT=tJk[:, 0:128], rhs=tJk[:, 128:640],
            start=True, stop=True,
        )._wait_ge(qmP, mjk)
        nc.tensor.matmul(
            psJb[:], lhsT=tJk[:, 0:128], rhs=tJk[:, 128:640],
            start=True, stop=True,
        )._wait_ge(qmP, mjk)

        # --- DVE: zero-pad the fp32r Cb columns 64:128 once, up front
        # (fp32 memset into scratch, then a rounding copy into the f32r
        # tile; a direct f32r memset fails the ISA check) ---
        qp_inc(nc.vector.memset(tZ[:], 0.0))
        mz = nqp[0]
        qp_inc(nc.vector.tensor_copy(out=tCbP[:, PPOS:128], in_=tZ[:])._wait_ge(qmP, mz))

        # --- scan rounds r=1..6 (w = 2^(r-1)) ---
        cpq_at = {}  # round -> qmP count of its pair copy
        cps_at = {}  # round -> qmS count of its S copy
        mm_at = {}  # tag -> pe count
        for r in range(1, 7):
            w = 1 << (r - 1)
            prev = pair(r - 1)
            tQ = prev[:, 0:17]
            tM = prev[:, 17:34]
            po = 34 * (r - 1)
            soff = w - 1
            mq = pe_inc(
                nc.tensor.matmul(
                    psQM[:, po : po + 17], lhsT=tM, rhs=tQ, start=True, stop=True
                )
            )
            mm = pe_inc(
                nc.tensor.matmul(
                    psQM[:, po + 17 : po + 34], lhsT=tQ, rhs=tM, start=True, stop=True
                )
            )
            mm_at[("m", r)] = npe[0]
            ms = pe_inc(
                nc.tensor.matmul(
                    psS[:, soff : soff + w],
                    lhsT=tQ,
                    rhs=tAll[:, C_S : C_S + w],
                    start=True,
                    stop=True,
                )
            )
            mm_at[("s", r)] = npe[0]
            if r == 1:
                mq._wait_ge(dmaIn, 16)
                mm._wait_ge(dmaIn, 16)
                ms._wait_ge(dmaIn, 16)
            else:
                # mmM/mmS inherit the pair dependency through PE program
                # order after mmQ's wait; mmS only needs the S-copy edge
                mq._wait_ge(qmP, cpq_at[r - 1])
                ms._wait_ge(qmS, cps_at[r - 1])
            qp_inc(
                nc.vector.tensor_copy(
                    out=pair(r)[:], in_=psQM[:, po : po + 34]
                )._wait_ge(pe, mm_at[("m", r)])
            )
            cpq_at[r] = nqp[0]
            qs_inc(
                nc.scalar.copy(
                    out=tAll[:, C_S + w : C_S + 2 * w], in_=psS[:, soff : soff + w]
                )._wait_ge(pe, mm_at[("s", r)])
            )
            cps_at[r] = nqs[0]

        # --- r7: [Q128 | M128] (no S extension) ---
        p6 = pair(6)
        pe_inc(
            nc.tensor.matmul(
                psQM[:, 204:221], lhsT=p6[:, 17:34], rhs=p6[:, 0:17],
                start=True, stop=True,
            )._wait_ge(qmP, cpq_at[6])
        )
        pe_inc(
            nc.tensor.matmul(
                psQM[:, 221:238], lhsT=p6[:, 0:17], rhs=p6[:, 17:34],
                start=True, stop=True,
            )
        )
        mm_at["r7"] = npe[0]

        # --- binary-decomposed jump: M_{64q} = M256^b2 M128^b1 M64^b0.
        # apply-k's matmul hides in the PE idle window of the next power
        # round, so only apply-3 + projection trail the power chain. ---
        tJ1 = tAll[:, C_JAT : C_JAT + 17]
        tJ2 = tAll[:, C_JBT : C_JBT + 17]
        tJ3 = tAll[:, C_BA3 : C_BA3 + 17]

        # blend1 (J1 in {I, M64}; data = Q64 from pair6)
        b1 = nc.vector.copy_predicated(
            out=tJ1,
            mask=tAll[:, C_BA1 : C_BA1 + 17].bitcast(mybir.dt.uint32),
            data=p6[:, 0:17],
        )._wait_ge(qmP, cpq_at[6])
        b1.wait_op(dmaInR, 16, "sem-ge", check=False)
        qp_inc(b1)
        bl1 = nqp[0]

        # apply-1: C1 = J1 S64 (runs in r7's shadow)
        a1 = pe_inc(
            nc.tensor.matmul(
                psC1[:], lhsT=tJ1, rhs=tAll[:, C_S : C_S + PPOS],
                start=True, stop=True,
            )
        )
        a1._wait_ge(qmP, bl1)
        a1.wait_op(qmS, cps_at[6], "sem-ge", check=False)  # S64 complete
        mm_at["a1"] = npe[0]

        qp_inc(
            nc.vector.tensor_copy(out=pair(7)[:], in_=psQM[:, 204:238])._wait_ge(
                pe, mm_at["r7"]
            )
        )
        cpq7 = nqp[0]

        # blend2 (J2 in {I, M128}; data = Q128 from pair7)
        qp_inc(
            nc.vector.copy_predicated(
                out=tJ2,
                mask=tAll[:, C_BA2 : C_BA2 + 17].bitcast(mybir.dt.uint32),
                data=pair(7)[:, 0:17],
            )._wait_ge(qmP, cpq7)
        )
        bl2 = nqp[0]
        # cpC1 (ACT): C1 -> SBUF, off the DVE chain
        qs_inc(nc.scalar.copy(out=tCa[:], in_=psC1[:])._wait_ge(pe, mm_at["a1"]))
        cpc1 = nqs[0]

        # --- r8: Q256 = Q128 Q128 ---
        mq256 = pe_inc(
            nc.tensor.matmul(
                psQ256[:], lhsT=pair(7)[:, 17:34], rhs=pair(7)[:, 0:17],
                start=True, stop=True,
            )._wait_ge(qmP, cpq7)
        )
        mm_at["r8"] = npe[0]

        # apply-2: C2 = J2 C1 (runs in r8's shadow)
        a2 = pe_inc(
            nc.tensor.matmul(psC2[:], lhsT=tJ2, rhs=tCa[:], start=True, stop=True)
        )
        a2._wait_ge(qmP, bl2)
        a2.wait_op(qmS, cpc1, "sem-ge", check=False)
        mm_at["a2"] = npe[0]

        # blend3 (J3 in {I, M256}; data = Q256 read straight from PSUM)
        qp_inc(
            nc.vector.copy_predicated(
                out=tJ3,
                mask=tAll[:, C_BB : C_BB + 17].bitcast(mybir.dt.uint32),
                data=psQ256[:],
            )._wait_ge(pe, mm_at["r8"])
        )
        bl3 = nqp[0]
        # cpC2 (ACT): C2 -> SBUF
        qs_inc(nc.scalar.copy(out=tZ[:], in_=psC2[:])._wait_ge(pe, mm_at["a2"]))
        cpc2 = nqs[0]

        # apply-3: C3 = J3 C2
        a3 = pe_inc(
            nc.tensor.matmul(psC3[:], lhsT=tJ3, rhs=tZ[:], start=True, stop=True)
        )
        a3._wait_ge(qmP, bl3)
        a3.wait_op(qmS, cpc2, "sem-ge", check=False)
        mm_at["a3"] = npe[0]
        # DVE cast rounds C3 into the fp32r projection operand
        qp_inc(
            nc.vector.tensor_copy(out=tCbP[:, 0:PPOS], in_=psC3[:])._wait_ge(
                pe, mm_at["a3"]
            )
        )
        cpCb = nqp[0]

        # --- projection: psO = CbPad^T Wb (float32r, 256-wide moving) ---
        mp0 = pe_inc(
            nc.tensor.matmul(
                psOa[:], lhsT=tCbP[:], rhs=tWb[:, 0:256], start=True, stop=True
            )
        )
        mp0._wait_ge(qmP, cpCb)
        mp0.wait_op(dmaIn2, 16, "sem-ge", check=False)  # Wb present
        mm_at["p0"] = npe[0]
        pe_inc(
            nc.tensor.matmul(
                psOb[:], lhsT=tCbP[:], rhs=tWb[:, 256:512], start=True, stop=True
            )
        )
        mm_at["p1"] = npe[0]

        # --- output copies (separate PSUM tensors per engine) + DMA ---
        nc.scalar.copy(out=tOut[:, 0:256], in_=psOa[0:PPOS, :])._wait_ge(
            pe, mm_at["p0"]
        ).then_inc(outS, 1)
        nc.vector.tensor_copy(out=tOut[:, 256:512], in_=psOb[0:PPOS, :])._wait_ge(
            pe, mm_at["p1"]
        ).then_inc(outS, 1)
        HP = PPOS // 2
        nc.sync.dma_start(out=dOut[0:HP, :], in_=tOut[0:HP, :])._wait_ge(
            outS, 2
        ).then_inc(dmaO, 16)
        nc.scalar.dma_start(out=dOut[HP:PPOS, :], in_=tOut[HP:PPOS, :])._wait_ge(
            outS, 2
        ).then_inc(dmaO, 16)
        nc.sync.wait_ge(dmaO, 32)

    nc.compile()
    return nc


def get_nc():
    key = VARIANT
    if key not in _NC_CACHE:
        _NC_CACHE[key] = _build_nc_raw()
    return _NC_CACHE[key]


def make_in_maps(pos_initial, pos_transition, W, b):
    T = np.ascontiguousarray(pos_transition, dtype=np.float32)
    p = np.asarray(pos_initial, dtype=np.float32).reshape(K)

    M1 = np.zeros((H, H), dtype=np.float32)
    M1[0:K, 0:K] = T
    M1[0:K, K] = p
    M1[K, K] = 1.0
    I17 = np.eye(H, dtype=np.float32)
    ones = np.ones((H, H), dtype=np.float32)
    zeros = np.zeros((H, H), dtype=np.float32)
    wb = np.concatenate(
        [W.T.astype(np.float32), b.reshape(1, -1).astype(np.float32)], axis=0
    )
    s0 = np.concatenate([p, [1.0]]).astype(np.float32)

    in_maps = []
    for c in range(NCORES):
        q = (NCORES - 1) - c
        inp0 = np.zeros((H, NCOL_IN0), dtype=np.float32)
        inp0[:, 0:17] = M1.T
        inp0[:, 17:34] = M1
        inp0[:, 34] = s0
        inp1 = np.zeros((H, NCOL_RST), dtype=np.float32)
        inp1[:, C_JAT - C_REST : C_JAT - C_REST + 17] = I17
        inp1[:, C_JBT - C_REST : C_JBT - C_REST + 17] = I17
        inp1[:, C_BA3 - C_REST : C_BA3 - C_REST + 17] = I17  # J3 identity
        inp1[:, C_BA1 - C_REST : C_BA1 - C_REST + 17] = ones if q & 1 else zeros
        inp1[:, C_BA2 - C_REST : C_BA2 - C_REST + 17] = ones if q & 2 else zeros
        inp1[:, C_BB - C_REST : C_BB - C_REST + 17] = ones if q & 4 else zeros
        in_maps.append(
            {
                "inp0": np.ascontiguousarray(inp0),
                "inp1": np.ascontiguousarray(inp1),
                "inp2": np.ascontiguousarray(wb),
            }
        )
    return in_maps


def assemble_output(per_core_results):
    out = np.empty((N, D), dtype=np.float32)
    for c in range(NCORES):
        arr = np.asarray(per_core_results[c]["out"])  # [64, 512]
        out[PPOS * c : PPOS * (c + 1), :] = arr[::-1, :]
    return out


def kernel(**inputs):
    pos_initial = np.asarray(inputs["pos_initial"], dtype=np.float32)
    pos_transition = np.asarray(inputs["pos_transition"], dtype=np.float32)
    W = np.asarray(inputs["W"], dtype=np.float32)
    b = np.asarray(inputs["b"], dtype=np.float32)
    n = int(inputs["sentence_len"])

    if n != N or pos_initial.shape[0] != K or W.shape != (D, K):
        return _host_fallback(pos_initial, pos_transition, W, b, n)

    from concourse.bass_utils import run_bass_kernel_spmd

    nc = get_nc()
    in_maps = make_in_maps(pos_initial, pos_transition, W, b)
    kwargs = {"trace": True} if TRACE else {}
    res = run_bass_kernel_spmd(nc, in_maps, core_ids=list(range(NCORES)), **kwargs)
    global LAST_RESULT
    LAST_RESULT = res
    return assemble_output(res.results)


if __name__ == "__main__":
    rng = np.random.default_rng(0)
    p = rng.normal(size=(K, 1)).astype(np.float32)
    A = rng.normal(size=(K, K)).astype(np.float32)
    q, r = np.linalg.qr(A)
    T = (q * np.sign(np.diag(r))[None, :]).astype(np.float32)
    W = rng.uniform(-0.25, 0.25, size=(D, K)).astype(np.float32)
    b = rng.uniform(-0.25, 0.25, size=(D,)).astype(np.float32)
    ref = _host_fallback(p, T, W, b, N)
    act = kernel(pos_initial=p, pos_transition=T, W=W, b=b, sentence_len=N)
    err = np.abs(act - ref).max() / np.abs(ref).max()
    print("max rel err vs host closed form:", err)
